# revision 11
# baseline (speedup 1.0000x reference)
"""Trainium2 Bass kernel for the 6-layer dense transformer encoder LM.

Sharding (8 NeuronCores, one trn2 chip):
  - Layers: sequence-parallel. Core c owns T=256 tokens (batch c//4, seq rows
    (c%4)*256..). Q/attention/FFN/LN run only on the core's own tokens; K,V are
    computed for own tokens then AllGather'd across the 4-core batch group.
  - Final d_model->vocab projection: vocab-sharded (4000 cols/core) over all
    2048 tokens (full hidden states AllGather'd once); log-softmax denominator
    via AllReduce of per-shard sum-exp.

Layout: activations are kept TRANSPOSED on chip ([feature, token]: features on
SBUF partitions, tokens on the free dim) so chained matmuls need no
transposes: Y^T = matmul(lhsT=W_ktile, rhs=X^T_ktile) accumulated over k.
LayerNorm stats (over features = partitions) use ones-vector matmuls on the PE;
per-token stats are partition-broadcast via GpSimd. Softmax runs on transposed
scores (keys on partitions): exp on ScalarE, the key-sum folded into the
probs@V matmul via an all-ones column appended to V.

Data movement: all DRAM-side tensors use partition-major layouts ([128, ...])
so every transfer is one large DMA with >=3KB contiguous runs. DMA queues are
assigned to spread issue cost and avoid head-of-line blocking: weight streams
on SyncE's HWDGE, collective-dependent loads on GpSimd's SWDGE, stores on the
producing engine's queue.
"""

import sys

sys.path.insert(0, "/opt/trn_rl_repo")

from dataclasses import dataclass

import numpy as np
import ml_dtypes

import concourse.bass as bass
import concourse.bacc as bacc
import concourse.mybir as mybir
import concourse.tile as tile
from concourse.bass import ts, ds
from concourse import bass_utils

F32 = mybir.dt.float32
BF16 = mybir.dt.bfloat16
F16 = mybir.dt.float16
AF = mybir.ActivationFunctionType
OP = mybir.AluOpType


@dataclass
class Cfg:
    B: int = 2
    S: int = 1024
    V: int = 32000
    D: int = 768
    F: int = 3072
    H: int = 12
    L: int = 6
    NC: int = 8
    SEG: int = 8  # final-stage token groups (one sum-exp AllReduce each)
    act_fn: object = AF.Gelu  # swapped to Identity for CoreSim tests (no Gelu in sim)

    def __post_init__(self):
        self.DH = self.D // self.H
        self.G = self.NC // self.B          # cores per batch group
        self.T = self.S // self.G           # tokens per core
        self.KD = self.D // 128             # d_model k-tiles
        self.KF = self.F // 128             # ffn k-tiles
        self.SK = self.S // 128             # key chunks
        self.TC = self.T // 128             # own-token chunks
        self.VS = self.V // self.NC         # vocab shard
        self.BT = self.B * self.S // 128    # total token chunks (final stage)
        self.DH1 = self.DH + 1
        # ffn1 weight blocking: k-tiles of W1 streamed in column blocks
        self.FBW = 1024 if self.F % 1024 == 0 else self.F
        self.FB = self.F // self.FBW
        # ffn2: W2 k-tiles streamed with halved columns
        self.DW = self.D // 2
        assert self.DW <= 512
        # final matmul n-splits
        self.NV = max(1, (self.VS + 511) // 512)
        assert self.VS % self.NV == 0
        self.NS = self.VS // self.NV
        assert self.NS <= 512
        assert self.D % 128 == 0 and self.F % 128 == 0 and self.T % 128 == 0
        assert self.DH == 64
        assert self.B * self.S % 128 == 0 and self.BT % self.SEG == 0


FULL = Cfg()


def build_program(cfg: Cfg, reps: int = 1, fake_collectives: bool = False,
                  skip_final: bool = False):
    """Build the SPMD Bass program (same program on all cores)."""
    c = cfg
    nc = bacc.Bacc("TRN2", target_bir_lowering=False, debug=False)

    # ---- DRAM I/O (partition-major layouts) ----
    embp = nc.dram_tensor("embp", [128, c.KD, c.T], F32, kind="ExternalInput")
    posp = nc.dram_tensor("posp", [128, c.KD, c.T], F32, kind="ExternalInput")
    wqr = nc.dram_tensor("wqr", [c.L, 128, c.KD, c.D], BF16, kind="ExternalInput")
    wkr = nc.dram_tensor("wkr", [c.L, 128, c.KD, c.D], BF16, kind="ExternalInput")
    wvr = nc.dram_tensor("wvr", [c.L, 128, c.KD, c.D], BF16, kind="ExternalInput")
    wor = nc.dram_tensor("wor", [c.L, 128, c.KD, c.D], BF16, kind="ExternalInput")
    w1r = nc.dram_tensor("w1r", [c.L, c.FB, 128, c.KD, c.FBW], BF16, kind="ExternalInput")
    w2r = nc.dram_tensor("w2r", [c.L, 2, 128, c.KF, c.DW], BF16, kind="ExternalInput")
    wfr = nc.dram_tensor("wfr", [128, c.KD, c.VS], BF16, kind="ExternalInput")
    lp_out = nc.dram_tensor("lp_out", [c.B * c.S, c.VS], F32, kind="ExternalOutput")
    hsetT_out = nc.dram_tensor("hsetT_out", [128, c.KD, c.T], F32, kind="ExternalOutput")

    groups_b = [list(range(b * c.G, (b + 1) * c.G)) for b in range(c.B)]
    groups_all = [list(range(c.NC))]

    KSZ = c.D * c.T                      # KT_own flat size
    VSZ = c.T * c.H * c.DH1              # V_own (ones-augmented) flat size
    HSZ = c.D * c.T                      # final hidden flat size

    qscale = 1.0 / float(np.sqrt(np.float32(c.D)))

    def kview(flat):   # [128, KD, T] partition-major view of a K region
        return flat.rearrange("(p ko t) -> p ko t", p=128, ko=c.KD)

    def vview(flat):   # [128, TC, H, DH1] view of a V region
        return flat.rearrange("(p tc h d) -> p tc h d", p=128, tc=c.TC, h=c.H)

    def do_collective(kind, op, in_t, out_t, rgs):
        if not fake_collectives:
            nc.gpsimd.collective_compute(
                kind, op, ins=[in_t.opt()], outs=[out_t.opt()], replica_groups=rgs
            )
        elif kind == "AllGather":
            n = out_t.shape[0]
            for r in range(n):
                nc.gpsimd.dma_start(out_t[r], in_t[:])
        else:  # AllReduce
            nc.gpsimd.dma_start(out_t[:], in_t[:])

    with tile.TileContext(nc) as tc:
        with (
            tc.tile_pool(name="const", bufs=1) as constp,
            tc.tile_pool(name="resid", bufs=3) as residp,
            tc.tile_pool(name="hTb", bufs=2) as hTbp,
            tc.tile_pool(name="small", bufs=6) as smallp,   # [1,T]-ish scratch
            tc.tile_pool(name="med", bufs=3) as medp,       # [128,T] f32 scratch
            tc.tile_pool(name="psmm", bufs=2, space="PSUM") as psmm,
            tc.tile_pool(name="pssc", bufs=2, space="PSUM") as pssc,
            tc.tile_pool(name="dram", bufs=2, space="DRAM") as dramp,
        ):
            ones128 = constp.tile([128, 1], F32, tag="ones")
            nc.gpsimd.memset(ones128[:], 1.0)
            eps1 = constp.tile([1, 1], F32, tag="eps")
            nc.gpsimd.memset(eps1[:], 1e-5)
            ones_row = constp.tile([1, 128], F32, tag="onesr")
            nc.gpsimd.memset(ones_row[:], 1.0)

            def layer_norm(x_in, out_f32, out_b16):
                """LN over features (partitions). tiles: [128, KD, T]."""
                sum_ps = psmm.tile([1, c.T], F32, tag="mm")
                sq_ps = psmm.tile([1, c.T], F32, tag="mm")
                for j in range(c.KD):
                    nc.tensor.matmul(
                        sum_ps[:], ones128[:], x_in[:, j, :],
                        start=(j == 0), stop=(j == c.KD - 1),
                    )
                for j in range(c.KD):
                    sq = medp.tile([128, c.T], F32, tag="sq")
                    nc.vector.tensor_mul(sq[:], x_in[:, j, :], x_in[:, j, :])
                    nc.tensor.matmul(
                        sq_ps[:], ones128[:], sq[:],
                        start=(j == 0), stop=(j == c.KD - 1),
                    )
                mu = smallp.tile([1, c.T], F32, tag="st")
                var = smallp.tile([1, c.T], F32, tag="st")
                msq = smallp.tile([1, c.T], F32, tag="st")
                sd = smallp.tile([1, c.T], F32, tag="st")
                rstd = smallp.tile([1, c.T], F32, tag="st")
                cc_ = smallp.tile([1, c.T], F32, tag="st")
                nc.vector.tensor_scalar_mul(mu[:], sum_ps[:], 1.0 / c.D)
                nc.vector.tensor_scalar_mul(var[:], sq_ps[:], 1.0 / c.D)
                nc.vector.tensor_mul(msq[:], mu[:], mu[:])
                nc.vector.tensor_sub(var[:], var[:], msq[:])
                nc.scalar.activation(sd[:], var[:], AF.Sqrt, bias=eps1[:])
                nc.vector.reciprocal(rstd[:], sd[:])
                nc.vector.tensor_mul(cc_[:], mu[:], rstd[:])
                nc.vector.tensor_scalar_mul(cc_[:], cc_[:], -1.0)
                bc_ps = pssc.tile([128, 2, c.T], F32, tag="sc")
                ab = bc_ps[:, 0, :]
                cb = bc_ps[:, 1, :]
                nc.tensor.matmul(ab, ones_row[:], rstd[:], start=True, stop=True)
                nc.tensor.matmul(cb, ones_row[:], cc_[:], start=True, stop=True)
                for j in range(c.KD):
                    nc.vector.tensor_mul(out_f32[:, j, :], x_in[:, j, :], ab[:])
                    nc.vector.tensor_add(out_f32[:, j, :], out_f32[:, j, :], cb[:])
                    nc.vector.tensor_copy(out_b16[:, j, :], out_f32[:, j, :])

            for _rep in range(reps):
                # ---- embedding: resid = emb + pos (transposed, f32) ----
                resid = residp.tile([128, c.KD, c.T], F32, tag="resid")
                hTb = hTbp.tile([128, c.KD, c.T], BF16, tag="hTb")
                with tc.tile_pool(name="embpool", bufs=2) as embpool:
                    et = embpool.tile([128, c.KD, c.T], F32, tag="emb")
                    pt = embpool.tile([128, c.KD, c.T], F32, tag="emb")
                    nc.sync.dma_start(et[:], embp[:])
                    nc.sync.dma_start(pt[:], posp[:])
                    for j in range(c.KD):
                        nc.vector.tensor_add(resid[:, j, :], et[:, j, :], pt[:, j, :])
                        nc.vector.tensor_copy(hTb[:, j, :], resid[:, j, :])

                # ================= layers =================
                with (
                    tc.tile_pool(name="wts", bufs=3) as wtsp,
                    tc.tile_pool(name="w1p", bufs=2) as w1p,
                    tc.tile_pool(name="w2p", bufs=2) as w2p,
                    tc.tile_pool(name="lact", bufs=2) as lactp,
                    tc.tile_pool(name="big1", bufs=1) as big1p,
                ):
                    for l in range(c.L):
                        # ---- Q, K projections (transposed outputs) ----
                        def proj_T(w_dram, out_b16, scale):
                            wp = wtsp.tile([128, c.KD, c.D], BF16, tag="wproj")
                            nc.sync.dma_start(wp[:], w_dram[l])
                            for m in range(c.KD):
                                ps = psmm.tile([128, c.T], F32, tag="mm")
                                for j in range(c.KD):
                                    nc.tensor.matmul(
                                        ps[:], wp[:, j, ts(m, 128)], hTb[:, j, :],
                                        start=(j == 0), stop=(j == c.KD - 1),
                                    )
                                nc.scalar.activation(out_b16[:, m, :], ps[:], AF.Copy, scale=scale)

                        QTb = lactp.tile([128, c.KD, c.T], BF16, tag="qkvT")
                        KTb = lactp.tile([128, c.KD, c.T], BF16, tag="qkvT")
                        # K first so its AllGather overlaps the V/Q projections
                        proj_T(wkr, KTb, 1.0)
                        k_in = dramp.tile([KSZ], BF16, tag="kin")
                        k_out = dramp.tile([c.G, KSZ], BF16, tag="kout",
                                           addr_space="Shared" if c.G > 4 else "Local")
                        nc.scalar.dma_start(kview(k_in[:]), KTb[:])
                        do_collective("AllGather", OP.bypass, k_in, k_out, groups_b)

                        # ---- V projection (natural layout, ones-augmented per head) ----
                        wp_v = wtsp.tile([128, c.KD, c.D], BF16, tag="wproj")
                        nc.sync.dma_start(wp_v[:], wvr[l])
                        Vown = lactp.tile([128, c.TC, c.H, c.DH1], BF16, tag="vown")
                        nc.gpsimd.memset(Vown[:, :, :, c.DH:c.DH1], 1.0)
                        nsplits = [(0, min(512, c.D))]
                        if c.D > 512:
                            nsplits.append((512, c.D - 512))
                        for t in range(c.TC):
                            ps = psmm.tile([128, 1024], F32, tag="mm")
                            for (n0, nn) in nsplits:
                                for j in range(c.KD):
                                    nc.tensor.matmul(
                                        ps[:, n0:n0 + nn],
                                        hTb[:, j, ts(t, 128)],
                                        wp_v[:, j, n0:n0 + nn],
                                        start=(j == 0), stop=(j == c.KD - 1),
                                    )
                            nc.vector.tensor_copy(
                                Vown[:, t, :, 0:c.DH],
                                ps[:, 0:c.D].rearrange("p (h d) -> p h d", h=c.H),
                            )
                        v_in = dramp.tile([VSZ], BF16, tag="vin")
                        v_out = dramp.tile([c.G, VSZ], BF16, tag="vout",
                                           addr_space="Shared" if c.G > 4 else "Local")
                        nc.scalar.dma_start(vview(v_in[:]), Vown[:])
                        do_collective("AllGather", OP.bypass, v_in, v_out, groups_b)

                        # Q projection runs while the K/V AllGathers are in flight
                        proj_T(wqr, QTb, qscale)

                        KTg = big1p.tile([128, c.G, c.KD, c.T], BF16, tag="ktg")
                        Vaug = big1p.tile([128, c.G, c.TC, c.H, c.DH1], BF16, tag="vaug")
                        for r in range(c.G):
                            nc.gpsimd.dma_start(KTg[:, r], kview(k_out[r]))
                            nc.gpsimd.dma_start(Vaug[:, r], vview(v_out[r]))

                        # ---- attention, head by head ----
                        attnT = lactp.tile([128, c.KD, c.T], BF16, tag="attnT")
                        for h in range(c.H):
                            jq = (h * c.DH) // 128
                            po = (h * c.DH) % 128
                            expT = lactp.tile([128, c.SK, c.T], BF16, tag="expt")
                            for half in range(2):
                                sc = pssc.tile([128, c.SK // 2, c.T], F32, tag="sc")
                                for k2 in range(c.SK // 2):
                                    kc = half * (c.SK // 2) + k2
                                    nc.tensor.matmul(
                                        sc[:, k2, :],
                                        KTg[po:po + c.DH, kc // c.TC, jq, ts(kc % c.TC, 128)],
                                        QTb[po:po + c.DH, jq, :],
                                        start=True, stop=True,
                                    )
                                nc.scalar.activation(
                                    expT[:, ds(half * (c.SK // 2), c.SK // 2), :], sc[:], AF.Exp
                                )
                            pv = psmm.tile([c.DH1, c.T], F32, tag="mm")
                            for kc in range(c.SK):
                                nc.tensor.matmul(
                                    pv[:], Vaug[:, kc // c.TC, kc % c.TC, h, :],
                                    expT[:, kc, :],
                                    start=(kc == 0), stop=(kc == c.SK - 1),
                                )
                            rc = smallp.tile([1, c.T], F32, tag="rc")
                            nc.vector.reciprocal(rc[:], pv[c.DH:c.DH1, :])
                            rcb = medp.tile([128, c.T], F32, tag="rcb")
                            nc.gpsimd.partition_broadcast(rcb[:], rc[:])
                            if po == 0:
                                nc.vector.tensor_tensor(
                                    attnT[0:c.DH, jq, :], pv[0:c.DH, :], rcb[0:c.DH, :], OP.mult
                                )
                            else:
                                stg = medp.tile([c.DH, c.T], BF16, tag="astg")
                                nc.vector.tensor_tensor(
                                    stg[:], pv[0:c.DH, :], rcb[0:c.DH, :], OP.mult
                                )
                                nc.gpsimd.dma_start(attnT[po:po + c.DH, jq, :], stg[:])

                        # ---- output projection + residual + LN1 ----
                        wp_o = wtsp.tile([128, c.KD, c.D], BF16, tag="wproj")
                        nc.sync.dma_start(wp_o[:], wor[l])
                        r1 = residp.tile([128, c.KD, c.T], F32, tag="resid")
                        for m in range(c.KD):
                            ps = psmm.tile([128, c.T], F32, tag="mm")
                            for j in range(c.KD):
                                nc.tensor.matmul(
                                    ps[:], wp_o[:, j, ts(m, 128)], attnT[:, j, :],
                                    start=(j == 0), stop=(j == c.KD - 1),
                                )
                            nc.vector.tensor_add(r1[:, m, :], ps[:], resid[:, m, :])
                        h1 = residp.tile([128, c.KD, c.T], F32, tag="resid")
                        h1b = hTbp.tile([128, c.KD, c.T], BF16, tag="hTb")
                        layer_norm(r1, h1, h1b)

                        # ---- FFN1 + gelu ----
                        geluT = big1p.tile([128, c.KF, c.T], BF16, tag="gelu")
                        for fb in range(c.FB):
                            w1t = w1p.tile([128, c.KD, c.FBW], BF16, tag="w1")
                            nc.sync.dma_start(w1t[:], w1r[l, fb])
                            for m2 in range(c.FBW // 128):
                                m = fb * (c.FBW // 128) + m2
                                ps = psmm.tile([128, c.T], F32, tag="mm")
                                for j in range(c.KD):
                                    nc.tensor.matmul(
                                        ps[:], w1t[:, j, ts(m2, 128)], h1b[:, j, :],
                                        start=(j == 0), stop=(j == c.KD - 1),
                                    )
                                nc.scalar.activation(geluT[:, m, :], ps[:], c.act_fn)

                        # ---- FFN2 + residual + LN2 ----
                        r2 = residp.tile([128, c.KD, c.T], F32, tag="resid")
                        for hf in range(2):
                            w2t = w2p.tile([128, c.KF, c.DW], BF16, tag="w2")
                            nc.sync.dma_start(w2t[:], w2r[l, hf])
                            for m2 in range(c.DW // 128):
                                m = hf * (c.DW // 128) + m2
                                ps = psmm.tile([128, c.T], F32, tag="mm")
                                for j in range(c.KF):
                                    nc.tensor.matmul(
                                        ps[:], w2t[:, j, ts(m2, 128)], geluT[:, j, :],
                                        start=(j == 0), stop=(j == c.KF - 1),
                                    )
                                nc.vector.tensor_add(r2[:, m, :], ps[:], h1[:, m, :])
                        new_resid = residp.tile([128, c.KD, c.T], F32, tag="resid")
                        new_hb = hTbp.tile([128, c.KD, c.T], BF16, tag="hTb")
                        layer_norm(r2, new_resid, new_hb)
                        resid = new_resid
                        hTb = new_hb

                if skip_final:
                    continue
                # ================= final stage =================
                nc.gpsimd.dma_start(hsetT_out[:], resid[:])
                hf_in = dramp.tile([HSZ], BF16, tag="hfin")
                hf_out = dramp.tile([c.NC, HSZ], BF16, tag="hfout",
                                    addr_space="Local" if fake_collectives else "Shared")
                nc.gpsimd.dma_start(kview(hf_in[:]), hTb[:])
                do_collective("AllGather", OP.bypass, hf_in, hf_out, groups_all)

                with (
                    tc.tile_pool(name="fhgat", bufs=1) as fhgatp,
                    tc.tile_pool(name="fwf", bufs=1) as fwfp,
                    tc.tile_pool(name="flg", bufs=2 * (c.BT // c.SEG) + 1) as flgp,
                    tc.tile_pool(name="fsc", bufs=3) as fscp,
                    tc.tile_pool(name="fot", bufs=2) as fotp,
                ):
                    hgat = fhgatp.tile([128, c.NC, c.KD, c.T], BF16, tag="hgat")
                    for r in range(c.NC):
                        nc.gpsimd.dma_start(hgat[:, r], kview(hf_out[r]))
                    wf_t = fwfp.tile([128, c.KD, c.VS], BF16, tag="wfk")
                    nc.sync.dma_start(wf_t[:], wfr[:])

                    HC = c.BT // c.SEG  # token chunks per AR group
                    pending = None  # (seg, lgs, se_out) awaiting finalize

                    def finalize(seg, lgs, se_out):
                        seg_t = smallp.tile([128, HC], F32, tag="seg")
                        nc.gpsimd.dma_start(seg_t[:], se_out[:])
                        lnse = smallp.tile([128, HC], F32, tag="lnse")
                        nc.scalar.activation(lnse[:], seg_t[:], AF.Ln)
                        for c2 in range(HC):
                            ch = seg * HC + c2
                            ot = fotp.tile([128, c.NV, c.NS], F32, tag="ot")
                            for n in range(c.NV):
                                nc.vector.tensor_scalar(
                                    ot[:, n, :], lgs[c2][:, n, :], lnse[:, c2:c2 + 1], None,
                                    OP.subtract,
                                )
                            nc.gpsimd.dma_start(
                                lp_out[ts(ch, 128), :],
                                ot[:].rearrange("p nv ns -> p (nv ns)"),
                            )

                    for seg in range(c.SEG):
                        lgs = []
                        se_grp = smallp.tile([128, HC], F32, tag="seh")
                        for c2 in range(HC):
                            ch = seg * HC + c2
                            r, tcn = ch // c.TC, ch % c.TC
                            lg = flgp.tile([128, c.NV, c.NS], F16, tag="lgt")
                            sep = smallp.tile([128, c.NV], F32, tag="sep")
                            for n in range(c.NV):
                                ps = psmm.tile([128, c.NS], F32, tag="mm")
                                for j in range(c.KD):
                                    nc.tensor.matmul(
                                        ps[:], hgat[:, r, j, ts(tcn, 128)],
                                        wf_t[:, j, ts(n, c.NS)],
                                        start=(j == 0), stop=(j == c.KD - 1),
                                    )
                                scr = fscp.tile([128, c.NS], BF16, tag="scr")
                                nc.scalar.activation(
                                    scr[:], ps[:], AF.Exp, accum_out=sep[:, n:n + 1]
                                )
                                nc.vector.tensor_copy(lg[:, n, :], ps[:])
                            lgs.append(lg)
                            if c.NV > 1:
                                nc.vector.reduce_sum(
                                    se_grp[:, c2:c2 + 1], sep[:], axis=mybir.AxisListType.X
                                )
                            else:
                                nc.vector.tensor_copy(se_grp[:, c2:c2 + 1], sep[:])
                        se_in = dramp.tile([128, HC], F32, tag="sein")
                        se_out = dramp.tile([128, HC], F32, tag="seout",
                                            addr_space="Local" if fake_collectives else "Shared")
                        nc.gpsimd.dma_start(se_in[:], se_grp[:])
                        do_collective("AllReduce", OP.add, se_in, se_out, groups_all)
                        if pending is not None:
                            finalize(*pending)
                        pending = (seg, lgs, se_out)
                    finalize(*pending)

    return nc


# ---------------- host side ----------------

def _to_bf16(a):
    return np.ascontiguousarray(a.astype(ml_dtypes.bfloat16))


def shard_inputs(cfg: Cfg, inp: dict):
    c = cfg
    x = np.asarray(inp["x"]).astype(np.int64).reshape(-1)           # [B*S]
    tok = np.asarray(inp["tok_emb"], np.float32)
    pos = np.asarray(inp["pos_emb"], np.float32)

    def proj_pm(w):  # [L, D, D] -> [L, 128, KD, D]
        return _to_bf16(
            np.asarray(w, np.float32).reshape(c.L, c.KD, 128, c.D).transpose(0, 2, 1, 3)
        )

    wqr = proj_pm(inp["Wq"])
    wkr = proj_pm(inp["Wk"])
    wvr = proj_pm(inp["Wv"])
    wor = proj_pm(inp["Wo"])
    w1 = np.asarray(inp["W1"], np.float32)
    w2 = np.asarray(inp["W2"], np.float32)
    wfull = np.asarray(inp["Wf"], np.float32)

    # [L, FB, 128, KD, FBW]
    w1r = _to_bf16(
        w1.reshape(c.L, c.KD, 128, c.FB, c.FBW).transpose(0, 3, 2, 1, 4)
    )
    # [L, 2, 128, KF, DW]
    w2r = _to_bf16(
        w2.reshape(c.L, c.KF, 128, 2, c.DW).transpose(0, 3, 2, 1, 4)
    )

    for nm in ("bq", "bk", "bv", "bo", "b1", "b2", "bf", "ln1_b", "ln2_b"):
        assert not np.any(np.asarray(inp[nm])), f"nonzero {nm} not supported"
    for nm in ("ln1_g", "ln2_g"):
        assert np.all(np.asarray(inp[nm]) == 1.0), f"non-unit {nm} not supported"

    def pm(a):  # [T, D] f32 -> [128, KD, T] partition-major transposed
        return np.ascontiguousarray(
            a.T.reshape(c.KD, 128, c.T).transpose(1, 0, 2)
        )

    in_maps = []
    for core in range(c.NC):
        g0 = core * c.T
        s0 = (core % c.G) * c.T
        wfs = _to_bf16(wfull[:, core * c.VS:(core + 1) * c.VS])
        wfr_ = np.ascontiguousarray(
            wfs.reshape(c.KD, 128, c.VS).transpose(1, 0, 2)
        )
        in_maps.append({
            "embp": pm(tok[x[g0:g0 + c.T]]),
            "posp": pm(pos[s0:s0 + c.T]),
            "wqr": wqr, "wkr": wkr, "wvr": wvr, "wor": wor,
            "w1r": w1r, "w2r": w2r, "wfr": wfr_,
        })
    return in_maps


def assemble_outputs(cfg: Cfg, results):
    c = cfg
    lp = np.empty((c.B * c.S, c.V), np.float32)
    hs = np.empty((c.B * c.S, c.D), np.float32)
    for core in range(c.NC):
        lp[:, core * c.VS:(core + 1) * c.VS] = results[core]["lp_out"]
        hsT = results[core]["hsetT_out"]            # [128, KD, T]
        hs[core * c.T:(core + 1) * c.T, :] = (
            hsT.transpose(1, 0, 2).reshape(c.D, c.T).T
        )
    return (
        lp.reshape(c.B, c.S, c.V),
        hs.reshape(c.B, c.S, c.D),
    )


_PROGRAM_CACHE = {}


def _get_program(cfg: Cfg):
    key = "full"
    if key not in _PROGRAM_CACHE:
        nc = build_program(cfg)
        nc.finalize()
        _PROGRAM_CACHE[key] = nc
    return _PROGRAM_CACHE[key]


def kernel(**inputs):
    cfg = FULL
    nc = _get_program(cfg)
    in_maps = shard_inputs(cfg, inputs)
    res = bass_utils.run_bass_kernel_spmd(nc, in_maps, core_ids=list(range(cfg.NC)))
    return assemble_outputs(cfg, res.results)


# revision 12
# speedup vs baseline: 146.4701x; 146.4701x over previous
"""Trainium2 Bass kernel for the 6-layer dense transformer encoder LM.

Sharding (8 NeuronCores, one trn2 chip):
  - Layers: sequence-parallel. Core c owns T=256 tokens (batch c//4, seq rows
    (c%4)*256..). Q/attention/FFN/LN run only on the core's own tokens; K,V are
    computed for own tokens then AllGather'd across the 4-core batch group.
  - Final d_model->vocab projection: vocab-sharded (4000 cols/core) over all
    2048 tokens (full hidden states AllGather'd once); log-softmax denominator
    via AllReduce of per-shard sum-exp.

Layout: activations are kept TRANSPOSED on chip ([feature, token]: features on
SBUF partitions, tokens on the free dim) so chained matmuls need no
transposes: Y^T = matmul(lhsT=W_ktile, rhs=X^T_ktile) accumulated over k.
LayerNorm stats (over features = partitions) use ones-vector matmuls on the PE;
per-token stats are partition-broadcast via GpSimd. Softmax runs on transposed
scores (keys on partitions): exp on ScalarE, the key-sum folded into the
probs@V matmul via an all-ones column appended to V.

Data movement: all DRAM-side tensors use partition-major layouts ([128, ...])
so every transfer is one large DMA with >=3KB contiguous runs. DMA queues are
assigned to spread issue cost and avoid head-of-line blocking: weight streams
on SyncE's HWDGE, collective-dependent loads on GpSimd's SWDGE, stores on the
producing engine's queue.
"""

import sys

sys.path.insert(0, "/opt/trn_rl_repo")

from dataclasses import dataclass

import numpy as np
import ml_dtypes

import concourse.bass as bass
import concourse.bacc as bacc
import concourse.mybir as mybir
import concourse.tile as tile
from concourse.bass import ts, ds
from concourse import bass_utils

F32 = mybir.dt.float32
BF16 = mybir.dt.bfloat16
F16 = mybir.dt.float16
AF = mybir.ActivationFunctionType
OP = mybir.AluOpType


@dataclass
class Cfg:
    B: int = 2
    S: int = 1024
    V: int = 32000
    D: int = 768
    F: int = 3072
    H: int = 12
    L: int = 6
    NC: int = 8
    SEG: int = 8  # final-stage token groups (one sum-exp AllReduce each)
    act_fn: object = AF.Gelu  # swapped to Identity for CoreSim tests (no Gelu in sim)

    def __post_init__(self):
        self.DH = self.D // self.H
        self.G = self.NC // self.B          # cores per batch group
        self.T = self.S // self.G           # tokens per core
        self.KD = self.D // 128             # d_model k-tiles
        self.KF = self.F // 128             # ffn k-tiles
        self.SK = self.S // 128             # key chunks
        self.TC = self.T // 128             # own-token chunks
        self.VS = self.V // self.NC         # vocab shard
        self.BT = self.B * self.S // 128    # total token chunks (final stage)
        self.DH1 = self.DH + 1
        # ffn1 weight blocking: k-tiles of W1 streamed in column blocks
        self.FBW = 1024 if self.F % 1024 == 0 else self.F
        self.FB = self.F // self.FBW
        # ffn2: W2 k-tiles streamed with halved columns
        self.DW = self.D // 2
        assert self.DW <= 512
        # final matmul n-splits
        self.NV = max(1, (self.VS + 511) // 512)
        assert self.VS % self.NV == 0
        self.NS = self.VS // self.NV
        assert self.NS <= 512
        assert self.D % 128 == 0 and self.F % 128 == 0 and self.T % 128 == 0
        assert self.DH == 64
        assert self.B * self.S % 128 == 0 and self.BT % self.SEG == 0


FULL = Cfg()


def build_program(cfg: Cfg, reps: int = 1, fake_collectives: bool = False,
                  skip_final: bool = False):
    """Build the SPMD Bass program (same program on all cores)."""
    c = cfg
    nc = bacc.Bacc("TRN2", target_bir_lowering=False, debug=False)

    # ---- DRAM I/O (partition-major layouts) ----
    embp = nc.dram_tensor("embp", [128, c.KD, c.T], F32, kind="ExternalInput")
    posp = nc.dram_tensor("posp", [128, c.KD, c.T], F32, kind="ExternalInput")
    wqr = nc.dram_tensor("wqr", [c.L, 128, c.KD, c.D], BF16, kind="ExternalInput")
    wkr = nc.dram_tensor("wkr", [c.L, 128, c.KD, c.D], BF16, kind="ExternalInput")
    wvr = nc.dram_tensor("wvr", [c.L, 128, c.KD, c.D], BF16, kind="ExternalInput")
    wor = nc.dram_tensor("wor", [c.L, 128, c.KD, c.D], BF16, kind="ExternalInput")
    w1r = nc.dram_tensor("w1r", [c.L, c.FB, 128, c.KD, c.FBW], BF16, kind="ExternalInput")
    w2r = nc.dram_tensor("w2r", [c.L, 2, 128, c.KF, c.DW], BF16, kind="ExternalInput")
    wfr = nc.dram_tensor("wfr", [128, c.KD, c.VS], BF16, kind="ExternalInput")
    lp_out = nc.dram_tensor("lp_out", [c.B * c.S, c.VS], F32, kind="ExternalOutput")
    hsetT_out = nc.dram_tensor("hsetT_out", [128, c.KD, c.T], F32, kind="ExternalOutput")

    groups_b = [list(range(b * c.G, (b + 1) * c.G)) for b in range(c.B)]
    groups_all = [list(range(c.NC))]

    KSZ = c.D * c.T                      # KT_own flat size
    VSZ = c.T * c.H * c.DH1              # V_own (ones-augmented) flat size
    HSZ = c.D * c.T                      # final hidden flat size

    qscale = 1.0 / float(np.sqrt(np.float32(c.D)))

    def kview(flat):   # [128, KD, T] partition-major view of a K region
        return flat.rearrange("(p ko t) -> p ko t", p=128, ko=c.KD)

    def vview(flat):   # [128, TC, H, DH1] view of a V region
        return flat.rearrange("(p tc h d) -> p tc h d", p=128, tc=c.TC, h=c.H)

    def do_collective(kind, op, in_t, out_t, rgs):
        if not fake_collectives:
            nc.gpsimd.collective_compute(
                kind, op, ins=[in_t.opt()], outs=[out_t.opt()], replica_groups=rgs
            )
        elif kind == "AllGather":
            n = out_t.shape[0]
            for r in range(n):
                nc.gpsimd.dma_start(out_t[r], in_t[:])
        else:  # AllReduce
            nc.gpsimd.dma_start(out_t[:], in_t[:])

    with tile.TileContext(nc) as tc:
        with (
            tc.tile_pool(name="const", bufs=1) as constp,
            tc.tile_pool(name="resid", bufs=3) as residp,
            tc.tile_pool(name="hTb", bufs=2) as hTbp,
            tc.tile_pool(name="small", bufs=6) as smallp,   # [1,T]-ish scratch
            tc.tile_pool(name="med", bufs=3) as medp,       # [128,T] f32 scratch
            tc.tile_pool(name="psmm", bufs=4, space="PSUM") as psmm,
            tc.tile_pool(name="pssc", bufs=2, space="PSUM") as pssc,
            tc.tile_pool(name="dram", bufs=2, space="DRAM") as dramp,
        ):
            ones128 = constp.tile([128, 1], F32, tag="ones")
            nc.gpsimd.memset(ones128[:], 1.0)
            eps1 = constp.tile([1, 1], F32, tag="eps")
            nc.gpsimd.memset(eps1[:], 1e-5)
            ones_row = constp.tile([1, 128], F32, tag="onesr")
            nc.gpsimd.memset(ones_row[:], 1.0)

            def layer_norm(x_in, out_f32, out_b16):
                """LN over features (partitions). tiles: [128, KD, T]."""
                sum_ps = psmm.tile([1, c.T], F32, tag="mm")
                sq_ps = psmm.tile([1, c.T], F32, tag="mm")
                for j in range(c.KD):
                    nc.tensor.matmul(
                        sum_ps[:], ones128[:], x_in[:, j, :],
                        start=(j == 0), stop=(j == c.KD - 1),
                    )
                for j in range(c.KD):
                    sq = medp.tile([128, c.T], F32, tag="sq")
                    nc.vector.tensor_mul(sq[:], x_in[:, j, :], x_in[:, j, :])
                    nc.tensor.matmul(
                        sq_ps[:], ones128[:], sq[:],
                        start=(j == 0), stop=(j == c.KD - 1),
                    )
                mu = smallp.tile([1, c.T], F32, tag="st")
                var = smallp.tile([1, c.T], F32, tag="st")
                msq = smallp.tile([1, c.T], F32, tag="st")
                sd = smallp.tile([1, c.T], F32, tag="st")
                rstd = smallp.tile([1, c.T], F32, tag="st")
                cc_ = smallp.tile([1, c.T], F32, tag="st")
                nc.vector.tensor_scalar_mul(mu[:], sum_ps[:], 1.0 / c.D)
                nc.vector.tensor_scalar_mul(var[:], sq_ps[:], 1.0 / c.D)
                nc.vector.tensor_mul(msq[:], mu[:], mu[:])
                nc.vector.tensor_sub(var[:], var[:], msq[:])
                nc.scalar.activation(sd[:], var[:], AF.Sqrt, bias=eps1[:])
                nc.vector.reciprocal(rstd[:], sd[:])
                nc.vector.tensor_mul(cc_[:], mu[:], rstd[:])
                nc.vector.tensor_scalar_mul(cc_[:], cc_[:], -1.0)
                bc_ps = pssc.tile([128, 2, c.T], F32, tag="sc")
                ab = bc_ps[:, 0, :]
                cb = bc_ps[:, 1, :]
                nc.tensor.matmul(ab, ones_row[:], rstd[:], start=True, stop=True)
                nc.tensor.matmul(cb, ones_row[:], cc_[:], start=True, stop=True)
                for j in range(c.KD):
                    nc.vector.tensor_mul(out_f32[:, j, :], x_in[:, j, :], ab[:])
                    nc.vector.tensor_add(out_f32[:, j, :], out_f32[:, j, :], cb[:])
                    nc.vector.tensor_copy(out_b16[:, j, :], out_f32[:, j, :])

            for _rep in range(reps):
                # ---- embedding: resid = emb + pos (transposed, f32) ----
                resid = residp.tile([128, c.KD, c.T], F32, tag="resid")
                hTb = hTbp.tile([128, c.KD, c.T], BF16, tag="hTb")
                with tc.tile_pool(name="embpool", bufs=2) as embpool:
                    et = embpool.tile([128, c.KD, c.T], F32, tag="emb")
                    pt = embpool.tile([128, c.KD, c.T], F32, tag="emb")
                    nc.sync.dma_start(et[:], embp[:])
                    nc.sync.dma_start(pt[:], posp[:])
                    for j in range(c.KD):
                        nc.vector.tensor_add(resid[:, j, :], et[:, j, :], pt[:, j, :])
                        nc.vector.tensor_copy(hTb[:, j, :], resid[:, j, :])

                # ================= layers =================
                with (
                    tc.tile_pool(name="wts", bufs=3) as wtsp,
                    tc.tile_pool(name="w1p", bufs=2) as w1p,
                    tc.tile_pool(name="w2p", bufs=2) as w2p,
                    tc.tile_pool(name="lact", bufs=2) as lactp,
                    tc.tile_pool(name="big1", bufs=1) as big1p,
                ):
                    for l in range(c.L):
                        # ---- Q, K projections (transposed outputs) ----
                        def proj_T(w_dram, out_b16, scale):
                            wp = wtsp.tile([128, c.KD, c.D], BF16, tag="wproj")
                            nc.sync.dma_start(wp[:], w_dram[l])
                            for m in range(c.KD):
                                ps = psmm.tile([128, c.T], F32, tag="mm")
                                for j in range(c.KD):
                                    nc.tensor.matmul(
                                        ps[:], wp[:, j, ts(m, 128)], hTb[:, j, :],
                                        start=(j == 0), stop=(j == c.KD - 1),
                                    )
                                nc.scalar.activation(out_b16[:, m, :], ps[:], AF.Copy, scale=scale)

                        QTb = lactp.tile([128, c.KD, c.T], BF16, tag="qkvT")
                        KTb = lactp.tile([128, c.KD, c.T], BF16, tag="qkvT")
                        # K first so its AllGather overlaps the V/Q projections
                        proj_T(wkr, KTb, 1.0)
                        k_in = dramp.tile([KSZ], BF16, tag="kin")
                        k_out = dramp.tile([c.G, KSZ], BF16, tag="kout",
                                           addr_space="Shared" if c.G > 4 else "Local")
                        nc.scalar.dma_start(kview(k_in[:]), KTb[:])
                        do_collective("AllGather", OP.bypass, k_in, k_out, groups_b)

                        # ---- V projection (natural layout, ones-augmented per head) ----
                        wp_v = wtsp.tile([128, c.KD, c.D], BF16, tag="wproj")
                        nc.sync.dma_start(wp_v[:], wvr[l])
                        Vown = lactp.tile([128, c.TC, c.H, c.DH1], BF16, tag="vown")
                        nc.gpsimd.memset(Vown[:, :, :, c.DH:c.DH1], 1.0)
                        nsplits = [(0, min(512, c.D))]
                        if c.D > 512:
                            nsplits.append((512, c.D - 512))
                        for t in range(c.TC):
                            for (n0, nn) in nsplits:
                                ps = psmm.tile([128, nn], F32, tag="mm")
                                for j in range(c.KD):
                                    nc.tensor.matmul(
                                        ps[:],
                                        hTb[:, j, ts(t, 128)],
                                        wp_v[:, j, n0:n0 + nn],
                                        start=(j == 0), stop=(j == c.KD - 1),
                                    )
                                h0 = n0 // c.DH
                                nc.vector.tensor_copy(
                                    Vown[:, t, h0:h0 + nn // c.DH, 0:c.DH],
                                    ps[:].rearrange("p (h d) -> p h d", d=c.DH),
                                )
                        v_in = dramp.tile([VSZ], BF16, tag="vin")
                        v_out = dramp.tile([c.G, VSZ], BF16, tag="vout",
                                           addr_space="Shared" if c.G > 4 else "Local")
                        nc.scalar.dma_start(vview(v_in[:]), Vown[:])
                        do_collective("AllGather", OP.bypass, v_in, v_out, groups_b)

                        # Q projection runs while the K/V AllGathers are in flight
                        proj_T(wqr, QTb, qscale)

                        KTg = big1p.tile([128, c.G, c.KD, c.T], BF16, tag="ktg")
                        Vaug = big1p.tile([128, c.G, c.TC, c.H, c.DH1], BF16, tag="vaug")
                        for r in range(c.G):
                            nc.gpsimd.dma_start(KTg[:, r], kview(k_out[r]))
                            nc.gpsimd.dma_start(Vaug[:, r], vview(v_out[r]))

                        # ---- attention, head by head ----
                        attnT = lactp.tile([128, c.KD, c.T], BF16, tag="attnT")
                        for h in range(c.H):
                            jq = (h * c.DH) // 128
                            po = (h * c.DH) % 128
                            expT = lactp.tile([128, c.SK, c.T], BF16, tag="expt")
                            for half in range(2):
                                sc = pssc.tile([128, c.SK // 2, c.T], F32, tag="sc")
                                for k2 in range(c.SK // 2):
                                    kc = half * (c.SK // 2) + k2
                                    nc.tensor.matmul(
                                        sc[:, k2, :],
                                        KTg[po:po + c.DH, kc // c.TC, jq, ts(kc % c.TC, 128)],
                                        QTb[po:po + c.DH, jq, :],
                                        start=True, stop=True,
                                    )
                                nc.scalar.activation(
                                    expT[:, ds(half * (c.SK // 2), c.SK // 2), :], sc[:], AF.Exp
                                )
                            pv = psmm.tile([c.DH1, c.T], F32, tag="mm")
                            for kc in range(c.SK):
                                nc.tensor.matmul(
                                    pv[:], Vaug[:, kc // c.TC, kc % c.TC, h, :],
                                    expT[:, kc, :],
                                    start=(kc == 0), stop=(kc == c.SK - 1),
                                )
                            rc = smallp.tile([1, c.T], F32, tag="rc")
                            nc.vector.reciprocal(rc[:], pv[c.DH:c.DH1, :])
                            rcb = medp.tile([128, c.T], F32, tag="rcb")
                            nc.gpsimd.partition_broadcast(rcb[:], rc[:])
                            if po == 0:
                                nc.vector.tensor_tensor(
                                    attnT[0:c.DH, jq, :], pv[0:c.DH, :], rcb[0:c.DH, :], OP.mult
                                )
                            else:
                                stg = medp.tile([c.DH, c.T], BF16, tag="astg")
                                nc.vector.tensor_tensor(
                                    stg[:], pv[0:c.DH, :], rcb[0:c.DH, :], OP.mult
                                )
                                nc.gpsimd.dma_start(attnT[po:po + c.DH, jq, :], stg[:])

                        # ---- output projection + residual + LN1 ----
                        wp_o = wtsp.tile([128, c.KD, c.D], BF16, tag="wproj")
                        nc.sync.dma_start(wp_o[:], wor[l])
                        r1 = residp.tile([128, c.KD, c.T], F32, tag="resid")
                        for m in range(c.KD):
                            ps = psmm.tile([128, c.T], F32, tag="mm")
                            for j in range(c.KD):
                                nc.tensor.matmul(
                                    ps[:], wp_o[:, j, ts(m, 128)], attnT[:, j, :],
                                    start=(j == 0), stop=(j == c.KD - 1),
                                )
                            nc.vector.tensor_add(r1[:, m, :], ps[:], resid[:, m, :])
                        h1 = residp.tile([128, c.KD, c.T], F32, tag="resid")
                        h1b = hTbp.tile([128, c.KD, c.T], BF16, tag="hTb")
                        layer_norm(r1, h1, h1b)

                        # ---- FFN1 + gelu ----
                        geluT = big1p.tile([128, c.KF, c.T], BF16, tag="gelu")
                        for fb in range(c.FB):
                            w1t = w1p.tile([128, c.KD, c.FBW], BF16, tag="w1")
                            nc.sync.dma_start(w1t[:], w1r[l, fb])
                            for m2 in range(c.FBW // 128):
                                m = fb * (c.FBW // 128) + m2
                                ps = psmm.tile([128, c.T], F32, tag="mm")
                                for j in range(c.KD):
                                    nc.tensor.matmul(
                                        ps[:], w1t[:, j, ts(m2, 128)], h1b[:, j, :],
                                        start=(j == 0), stop=(j == c.KD - 1),
                                    )
                                nc.scalar.activation(geluT[:, m, :], ps[:], c.act_fn)

                        # ---- FFN2 + residual + LN2 ----
                        r2 = residp.tile([128, c.KD, c.T], F32, tag="resid")
                        for hf in range(2):
                            w2t = w2p.tile([128, c.KF, c.DW], BF16, tag="w2")
                            nc.sync.dma_start(w2t[:], w2r[l, hf])
                            for m2 in range(c.DW // 128):
                                m = hf * (c.DW // 128) + m2
                                ps = psmm.tile([128, c.T], F32, tag="mm")
                                for j in range(c.KF):
                                    nc.tensor.matmul(
                                        ps[:], w2t[:, j, ts(m2, 128)], geluT[:, j, :],
                                        start=(j == 0), stop=(j == c.KF - 1),
                                    )
                                nc.vector.tensor_add(r2[:, m, :], ps[:], h1[:, m, :])
                        new_resid = residp.tile([128, c.KD, c.T], F32, tag="resid")
                        new_hb = hTbp.tile([128, c.KD, c.T], BF16, tag="hTb")
                        layer_norm(r2, new_resid, new_hb)
                        resid = new_resid
                        hTb = new_hb

                if skip_final:
                    continue
                # ================= final stage =================
                nc.gpsimd.dma_start(hsetT_out[:], resid[:])
                hf_in = dramp.tile([HSZ], BF16, tag="hfin")
                hf_out = dramp.tile([c.NC, HSZ], BF16, tag="hfout",
                                    addr_space="Local" if fake_collectives else "Shared")
                nc.gpsimd.dma_start(kview(hf_in[:]), hTb[:])
                do_collective("AllGather", OP.bypass, hf_in, hf_out, groups_all)

                with (
                    tc.tile_pool(name="fhgat", bufs=1) as fhgatp,
                    tc.tile_pool(name="fwf", bufs=1) as fwfp,
                    tc.tile_pool(name="flg", bufs=2 * (c.BT // c.SEG) + 1) as flgp,
                    tc.tile_pool(name="fsc", bufs=3) as fscp,
                    tc.tile_pool(name="fot", bufs=2) as fotp,
                ):
                    hgat = fhgatp.tile([128, c.NC, c.KD, c.T], BF16, tag="hgat")
                    for r in range(c.NC):
                        nc.gpsimd.dma_start(hgat[:, r], kview(hf_out[r]))
                    wf_t = fwfp.tile([128, c.KD, c.VS], BF16, tag="wfk")
                    nc.sync.dma_start(wf_t[:], wfr[:])

                    HC = c.BT // c.SEG  # token chunks per AR group
                    pending = None  # (seg, lgs, se_out) awaiting finalize

                    def finalize(seg, lgs, se_out):
                        seg_t = smallp.tile([128, HC], F32, tag="seg")
                        nc.gpsimd.dma_start(seg_t[:], se_out[:])
                        lnse = smallp.tile([128, HC], F32, tag="lnse")
                        nc.scalar.activation(lnse[:], seg_t[:], AF.Ln)
                        for c2 in range(HC):
                            ch = seg * HC + c2
                            ot = fotp.tile([128, c.NV, c.NS], F32, tag="ot")
                            for n in range(c.NV):
                                nc.vector.tensor_scalar(
                                    ot[:, n, :], lgs[c2][:, n, :], lnse[:, c2:c2 + 1], None,
                                    OP.subtract,
                                )
                            nc.gpsimd.dma_start(
                                lp_out[ts(ch, 128), :],
                                ot[:].rearrange("p nv ns -> p (nv ns)"),
                            )

                    for seg in range(c.SEG):
                        lgs = []
                        se_grp = smallp.tile([128, HC], F32, tag="seh")
                        for c2 in range(HC):
                            ch = seg * HC + c2
                            r, tcn = ch // c.TC, ch % c.TC
                            lg = flgp.tile([128, c.NV, c.NS], F16, tag="lgt")
                            sep = smallp.tile([128, c.NV], F32, tag="sep")
                            for n in range(c.NV):
                                ps = psmm.tile([128, c.NS], F32, tag="mm")
                                for j in range(c.KD):
                                    nc.tensor.matmul(
                                        ps[:], hgat[:, r, j, ts(tcn, 128)],
                                        wf_t[:, j, ts(n, c.NS)],
                                        start=(j == 0), stop=(j == c.KD - 1),
                                    )
                                scr = fscp.tile([128, c.NS], BF16, tag="scr")
                                nc.scalar.activation(
                                    scr[:], ps[:], AF.Exp, accum_out=sep[:, n:n + 1]
                                )
                                nc.vector.tensor_copy(lg[:, n, :], ps[:])
                            lgs.append(lg)
                            if c.NV > 1:
                                nc.vector.reduce_sum(
                                    se_grp[:, c2:c2 + 1], sep[:], axis=mybir.AxisListType.X
                                )
                            else:
                                nc.vector.tensor_copy(se_grp[:, c2:c2 + 1], sep[:])
                        se_in = dramp.tile([128, HC], F32, tag="sein")
                        se_out = dramp.tile([128, HC], F32, tag="seout",
                                            addr_space="Local" if fake_collectives else "Shared")
                        nc.gpsimd.dma_start(se_in[:], se_grp[:])
                        do_collective("AllReduce", OP.add, se_in, se_out, groups_all)
                        if pending is not None:
                            finalize(*pending)
                        pending = (seg, lgs, se_out)
                    finalize(*pending)

    return nc


# ---------------- host side ----------------

def _to_bf16(a):
    return np.ascontiguousarray(a.astype(ml_dtypes.bfloat16))


def shard_inputs(cfg: Cfg, inp: dict):
    c = cfg
    x = np.asarray(inp["x"]).astype(np.int64).reshape(-1)           # [B*S]
    tok = np.asarray(inp["tok_emb"], np.float32)
    pos = np.asarray(inp["pos_emb"], np.float32)

    def proj_pm(w):  # [L, D, D] -> [L, 128, KD, D]
        return _to_bf16(
            np.asarray(w, np.float32).reshape(c.L, c.KD, 128, c.D).transpose(0, 2, 1, 3)
        )

    wqr = proj_pm(inp["Wq"])
    wkr = proj_pm(inp["Wk"])
    wvr = proj_pm(inp["Wv"])
    wor = proj_pm(inp["Wo"])
    w1 = np.asarray(inp["W1"], np.float32)
    w2 = np.asarray(inp["W2"], np.float32)
    wfull = np.asarray(inp["Wf"], np.float32)

    # [L, FB, 128, KD, FBW]
    w1r = _to_bf16(
        w1.reshape(c.L, c.KD, 128, c.FB, c.FBW).transpose(0, 3, 2, 1, 4)
    )
    # [L, 2, 128, KF, DW]
    w2r = _to_bf16(
        w2.reshape(c.L, c.KF, 128, 2, c.DW).transpose(0, 3, 2, 1, 4)
    )

    for nm in ("bq", "bk", "bv", "bo", "b1", "b2", "bf", "ln1_b", "ln2_b"):
        assert not np.any(np.asarray(inp[nm])), f"nonzero {nm} not supported"
    for nm in ("ln1_g", "ln2_g"):
        assert np.all(np.asarray(inp[nm]) == 1.0), f"non-unit {nm} not supported"

    def pm(a):  # [T, D] f32 -> [128, KD, T] partition-major transposed
        return np.ascontiguousarray(
            a.T.reshape(c.KD, 128, c.T).transpose(1, 0, 2)
        )

    in_maps = []
    for core in range(c.NC):
        g0 = core * c.T
        s0 = (core % c.G) * c.T
        wfs = _to_bf16(wfull[:, core * c.VS:(core + 1) * c.VS])
        wfr_ = np.ascontiguousarray(
            wfs.reshape(c.KD, 128, c.VS).transpose(1, 0, 2)
        )
        in_maps.append({
            "embp": pm(tok[x[g0:g0 + c.T]]),
            "posp": pm(pos[s0:s0 + c.T]),
            "wqr": wqr, "wkr": wkr, "wvr": wvr, "wor": wor,
            "w1r": w1r, "w2r": w2r, "wfr": wfr_,
        })
    return in_maps


def assemble_outputs(cfg: Cfg, results):
    c = cfg
    lp = np.empty((c.B * c.S, c.V), np.float32)
    hs = np.empty((c.B * c.S, c.D), np.float32)
    for core in range(c.NC):
        lp[:, core * c.VS:(core + 1) * c.VS] = results[core]["lp_out"]
        hsT = results[core]["hsetT_out"]            # [128, KD, T]
        hs[core * c.T:(core + 1) * c.T, :] = (
            hsT.transpose(1, 0, 2).reshape(c.D, c.T).T
        )
    return (
        lp.reshape(c.B, c.S, c.V),
        hs.reshape(c.B, c.S, c.D),
    )


_PROGRAM_CACHE = {}


def _get_program(cfg: Cfg):
    key = "full"
    if key not in _PROGRAM_CACHE:
        nc = build_program(cfg)
        nc.finalize()
        _PROGRAM_CACHE[key] = nc
    return _PROGRAM_CACHE[key]


def kernel(**inputs):
    cfg = FULL
    nc = _get_program(cfg)
    in_maps = shard_inputs(cfg, inputs)
    res = bass_utils.run_bass_kernel_spmd(nc, in_maps, core_ids=list(range(cfg.NC)))
    return assemble_outputs(cfg, res.results)
